# revision 5
# baseline (speedup 1.0000x reference)
"""Trainium2 Bass kernel for MllamaTextSelfAttention (B=1, S=2048, HID=4096,
32 Q heads / 8 KV heads, HD=128, RoPE, causal mask, GQA).

Sharding: tensor-parallel over heads across 8 NeuronCores. Core c computes
Q heads [4c, 4c+4) and KV head c, plus the matching slice of the output
projection; the 8 partial outputs are summed on the host.

Device dataflow (per core, all matmuls in fp32r):
  - qT/kT/vT = W^T-stationary projections -> [d, s] layouts
  - RoPE applied in [d, s] layout (partition-shifted rotate-half)
  - V transposed to natural [s, d] via PE transpose
  - S^T[k, q] = kT-tile.T @ qT-stripe  (k-partition layout)
  - E = exp(S^T + mask)  (no max subtraction; scores are O(10), mask -1e9
    underflows exp to exactly 0)
  - den[1, q] = ones.T @ E  (partition reduction on PE)
  - O^T[d, q] = V-tile.T @ E  accumulated over k tiles
  - O^T *= broadcast(1/den)  (rank-1 ones x recip matmul for the
    partition-direction broadcast)
  - Y[s, hid] = O^T-tiles.T @ woT  streamed to DRAM
"""

import math
import os
import sys

for _p in (
    "/opt/trn_rl_repo",
    "/root/.axon_site",
    "/root/.axon_site/_ro/trn_rl_repo",
    "/root/.axon_site/_ro/pypackages",
):
    if os.path.isdir(_p) and _p not in sys.path:
        sys.path.append(_p)

import numpy as np
from contextlib import ExitStack

import concourse.bass as bass
import concourse.tile as tile
from concourse import mybir
from concourse.bass_utils import run_bass_kernel_spmd
from concourse.masks import make_identity

F32 = mybir.dt.float32
FR = mybir.dt.float32r
ACTF = mybir.ActivationFunctionType

B, S, HID = 1, 2048, 4096
NH, NKV, HD = 32, 8, 128
NCORES = 8
QH = NH // NCORES          # 4 q heads per core
SS = 512                   # sequence stripe (matmul free dim)
NQS = S // SS              # 4 stripes
NKT = S // 128             # 16 k tiles
KH = HID // 128            # 32 hidden-dim k tiles
NEG = -1e9


def _split_multi_waits(nc: bass.Bass):
    """Walrus in this container encodes at most ONE sync-wait command per
    instruction. Hoist extra waits onto injected same-engine NoOps placed
    immediately before the instruction; engines are in-order so the
    semantics are unchanged."""
    n = 0
    for fn in nc.m.functions:
        for bb in fn.blocks:
            out = []
            for inst in bb.instructions:
                si = inst.sync_info
                if si is not None and si.on_wait and len(si.on_wait) > 1:
                    waits = list(si.on_wait)
                    for w in waits[:-1]:
                        n += 1
                        nop = mybir.InstNoOp(name=f"I-swait-{n}", ins=[], outs=[])
                        nop.engine = inst.engine
                        nop.sync_info = mybir.SyncInfo(on_wait=[w], on_update=[])
                        out.append(nop)
                    si.on_wait = [waits[-1]]
                out.append(inst)
            bb.instructions[:] = out
    return nc


_BUILD_CACHE = {}


def _build(causal: bool, split_waits: bool = True) -> bass.Bass:
    key = (causal, split_waits)
    if key in _BUILD_CACHE:
        return _BUILD_CACHE[key]

    nc = bass.Bass()
    hT = nc.dram_tensor("hT", [HID, S], F32, kind="ExternalInput")
    wqT = nc.dram_tensor("wqT", [HID, QH * HD], F32, kind="ExternalInput")
    wkT = nc.dram_tensor("wkT", [HID, HD], F32, kind="ExternalInput")
    wvT = nc.dram_tensor("wvT", [HID, HD], F32, kind="ExternalInput")
    woT = nc.dram_tensor("woT", [QH * HD, HID], F32, kind="ExternalInput")
    cosT = nc.dram_tensor("cosT", [HD, S], F32, kind="ExternalInput")
    sinT = nc.dram_tensor("sinT", [HD, S], F32, kind="ExternalInput")
    if causal:
        maskd = nc.dram_tensor("maskd", [128, 4 * SS], F32, kind="ExternalInput")
    else:
        maskT = nc.dram_tensor("maskT", [S, S], F32, kind="ExternalInput")
    y = nc.dram_tensor("y", [S, HID], F32, kind="ExternalOutput")

    with tile.TileContext(nc) as tc, ExitStack() as ctx:
        outer = ctx.enter_context(tc.tile_pool(name="outer", bufs=1))
        qT = outer.tile([128, QH * S], FR)      # [d, h*s]
        kT = outer.tile([128, S], FR)           # [d, s]
        vT = outer.tile([128, S], F32)          # [d, s]
        v_sb = outer.tile([128, S], FR)         # [s-within-tile, t*d]
        ot = outer.tile([128, QH * S], FR)      # [d, h*s] normalized O^T

        # ---------------- phase 1: QKV projections ----------------
        with (
            tc.tile_pool(name="wqkv", bufs=1) as wp,
            tc.tile_pool(name="hstream", bufs=3) as hp,
            tc.tile_pool(name="ps1", bufs=1, space="PSUM") as pp1,
        ):
            wq_sb = wp.tile([128, KH * QH * HD], FR)
            wk_sb = wp.tile([128, KH * HD], FR)
            wv_sb = wp.tile([128, KH * HD], FR)
            for k in range(KH):
                nc.sync.dma_start(
                    wq_sb[:, k * 512 : (k + 1) * 512],
                    wqT[k * 128 : (k + 1) * 128, :].bitcast(FR),
                )
                nc.sync.dma_start(
                    wk_sb[:, k * 128 : (k + 1) * 128],
                    wkT[k * 128 : (k + 1) * 128, :].bitcast(FR),
                )
                nc.sync.dma_start(
                    wv_sb[:, k * 128 : (k + 1) * 128],
                    wvT[k * 128 : (k + 1) * 128, :].bitcast(FR),
                )

            for n in range(NQS):
                psq = [
                    pp1.tile([128, SS], F32, name=f"psq{m}", tag=f"psq{m}")
                    for m in range(QH)
                ]
                psk = pp1.tile([128, SS], F32, tag="psk")
                psv = pp1.tile([128, SS], F32, tag="psv")
                for k in range(KH):
                    ht = hp.tile([128, SS], FR, tag="ht")
                    nc.sync.dma_start(
                        ht[:],
                        hT[k * 128 : (k + 1) * 128, n * SS : (n + 1) * SS].bitcast(FR),
                    )
                    st, sp = (k == 0), (k == KH - 1)
                    for m in range(QH):
                        nc.tensor.matmul(
                            psq[m][:],
                            wq_sb[:, k * 512 + m * 128 : k * 512 + (m + 1) * 128],
                            ht[:],
                            start=st,
                            stop=sp,
                        )
                    nc.tensor.matmul(
                        psk[:],
                        wk_sb[:, k * 128 : (k + 1) * 128],
                        ht[:],
                        start=st,
                        stop=sp,
                    )
                    nc.tensor.matmul(
                        psv[:],
                        wv_sb[:, k * 128 : (k + 1) * 128],
                        ht[:],
                        start=st,
                        stop=sp,
                    )
                for m in range(QH):
                    nc.scalar.copy(
                        qT[:, m * S + n * SS : m * S + (n + 1) * SS], psq[m][:]
                    )
                nc.scalar.copy(kT[:, n * SS : (n + 1) * SS], psk[:])
                nc.scalar.copy(vT[:, n * SS : (n + 1) * SS], psv[:])

        # ---------------- phase 1.5: RoPE + V transpose ----------------
        with (
            tc.tile_pool(name="rope", bufs=1) as rp,
            tc.tile_pool(name="pst", bufs=2, space="PSUM") as ppt,
        ):
            cos_sb = rp.tile([128, S], F32)
            sin_sb = rp.tile([128, S], F32)
            nc.sync.dma_start(cos_sb[:], cosT[:, :])
            nc.sync.dma_start(sin_sb[:], sinT[:, :])
            for i in range(QH + 1):
                src = qT[:, i * S : (i + 1) * S] if i < QH else kT[:, :]
                rot = rp.tile([128, S], F32, tag="rot")
                tmp = rp.tile([128, S], F32, tag="tmp")
                # rotate_half in [d, s]: rot[0:64] = -src[64:128]; rot[64:128] = src[0:64]
                nc.vector.tensor_scalar_mul(rot[0:64, :], src[64:128, :], -1.0)
                nc.vector.tensor_copy(rot[64:128, :], src[0:64, :])
                nc.vector.tensor_mul(tmp[:], src, cos_sb[:])
                nc.vector.tensor_mul(rot[:], rot[:], sin_sb[:])
                nc.vector.tensor_add(src, tmp[:], rot[:])

            id_sb = rp.tile([128, 128], F32)
            make_identity(nc, id_sb[:])
            for t in range(NKT):
                pst = ppt.tile([128, 128], F32, tag="pst")
                nc.tensor.transpose(pst[:], vT[:, t * 128 : (t + 1) * 128], id_sb[:])
                nc.scalar.copy(v_sb[:, t * 128 : (t + 1) * 128], pst[:])

        # ---------------- phase 2: attention ----------------
        with (
            tc.tile_pool(name="att", bufs=1) as ap_,
            tc.tile_pool(name="epool", bufs=2 if causal else 1) as ep,
            tc.tile_pool(name="mrowp", bufs=1) as mp,
            tc.tile_pool(name="ps2s", bufs=2, space="PSUM") as pp2s,
            tc.tile_pool(name="ps2a", bufs=1, space="PSUM") as pp2a,
        ):
            ones_f32 = ap_.tile([128, 128], F32)
            nc.vector.memset(ones_f32[:], 1.0)
            ones_sb = ap_.tile([128, 128], FR)
            nc.vector.tensor_copy(ones_sb[:], ones_f32[:])
            if causal:
                md_sb = ap_.tile([128, 4 * SS], F32)
                nc.sync.dma_start(md_sb[:], maskd[:, :])

            for qs in range(NQS):
                nkt = 4 * qs + 4 if causal else NKT
                if not causal:
                    mrow = mp.tile([128, NKT * SS], F32, tag="mrow")
                    for t in range(NKT):
                        nc.sync.dma_start(
                            mrow[:, t * SS : (t + 1) * SS],
                            maskT[t * 128 : (t + 1) * 128, qs * SS : (qs + 1) * SS],
                        )
                for h in range(QH):
                    e = ep.tile([128, NKT * SS], FR, tag="e")
                    qsl = qT[:, h * S + qs * SS : h * S + (qs + 1) * SS]
                    for t in range(nkt):
                        pss = pp2s.tile([128, SS], F32, tag="pss")
                        nc.tensor.matmul(
                            pss[:],
                            kT[:, t * 128 : (t + 1) * 128],
                            qsl,
                            start=True,
                            stop=True,
                        )
                        dst = e[:, t * SS : (t + 1) * SS]
                        if causal and t >= 4 * qs:
                            j = t - 4 * qs
                            nc.vector.tensor_add(
                                dst, pss[:], md_sb[:, j * SS : (j + 1) * SS]
                            )
                            nc.scalar.activation(dst, dst, ACTF.Exp)
                        elif not causal:
                            nc.vector.tensor_add(
                                dst, pss[:], mrow[:, t * SS : (t + 1) * SS]
                            )
                            nc.scalar.activation(dst, dst, ACTF.Exp)
                        else:
                            nc.scalar.activation(dst, pss[:], ACTF.Exp)

                    psd = pp2a.tile([1, SS], F32, tag="psd")
                    pso = pp2a.tile([128, SS], F32, tag="pso")
                    for t in range(nkt):
                        er = e[:, t * SS : (t + 1) * SS]
                        st, sp = (t == 0), (t == nkt - 1)
                        nc.tensor.matmul(
                            psd[:], ones_sb[:, 0:1], er, start=st, stop=sp
                        )
                        nc.tensor.matmul(
                            pso[:],
                            v_sb[:, t * 128 : (t + 1) * 128],
                            er,
                            start=st,
                            stop=sp,
                        )
                    den = ap_.tile([1, SS], F32, tag="den")
                    nc.scalar.copy(den[:], psd[:])
                    rec = ap_.tile([1, SS], FR, tag="rec")
                    with nc.allow_low_precision(reason="fp32r recip feeds matmul"):
                        nc.vector.reciprocal(rec[:], den[:])
                    psb = pp2a.tile([128, SS], F32, tag="psb")
                    nc.tensor.matmul(
                        psb[:],
                        ones_sb[0:1, 0:128],
                        rec[:],
                        start=True,
                        stop=True,
                    )
                    od = ot[:, h * S + qs * SS : h * S + (qs + 1) * SS]
                    nc.scalar.copy(od, pso[:])
                    nc.vector.tensor_mul(od, od, psb[:])

        # ---------------- phase 3: output projection ----------------
        with (
            tc.tile_pool(name="wop", bufs=1) as wop,
            tc.tile_pool(name="yout", bufs=3) as yp,
            tc.tile_pool(name="ps3", bufs=2, space="PSUM") as pp3,
        ):
            wo_sb = wop.tile([128, QH * HID], FR)
            for hh in range(QH):
                nc.sync.dma_start(
                    wo_sb[:, hh * HID : (hh + 1) * HID],
                    woT[hh * 128 : (hh + 1) * 128, :].bitcast(FR),
                )
            for st in range(NKT):
                for nn in range(HID // SS):
                    psy = pp3.tile([128, SS], F32, tag="psy")
                    for hh in range(QH):
                        nc.tensor.matmul(
                            psy[:],
                            ot[:, hh * S + st * 128 : hh * S + (st + 1) * 128],
                            wo_sb[:, hh * HID + nn * SS : hh * HID + (nn + 1) * SS],
                            start=(hh == 0),
                            stop=(hh == QH - 1),
                        )
                    yt = yp.tile([128, SS], F32, tag="yt")
                    nc.scalar.copy(yt[:], psy[:])
                    nc.sync.dma_start(
                        y[st * 128 : (st + 1) * 128, nn * SS : (nn + 1) * SS], yt[:]
                    )

    if split_waits:
        _split_multi_waits(nc)
    _BUILD_CACHE[key] = nc
    return nc


def _causal_mask_ref() -> np.ndarray:
    return np.triu(np.full((S, S), NEG, np.float32), k=1)


def _diag_mask_tiles() -> np.ndarray:
    p = np.arange(128, dtype=np.int64)[:, None]
    f = np.arange(SS, dtype=np.int64)[None, :]
    cols = [
        np.where(128 * j + p > f, np.float32(NEG), np.float32(0.0)) for j in range(4)
    ]
    return np.ascontiguousarray(np.concatenate(cols, axis=1).astype(np.float32))


def make_in_maps(hidden_states, attention_mask, cos, sin, wq, wk, wv, wo):
    """Host-side sharding/preprocessing. Returns (causal, in_maps)."""
    h = np.ascontiguousarray(np.asarray(hidden_states, dtype=np.float32)[0])
    m2 = np.ascontiguousarray(np.asarray(attention_mask, dtype=np.float32)[0, 0])
    wq = np.asarray(wq, dtype=np.float32)
    wk = np.asarray(wk, dtype=np.float32)
    wv = np.asarray(wv, dtype=np.float32)
    wo = np.asarray(wo, dtype=np.float32)

    causal = bool(np.array_equal(m2, _causal_mask_ref()))
    hT = np.ascontiguousarray(h.T)
    cosT = np.ascontiguousarray(np.asarray(cos, dtype=np.float32)[0].T)
    sinT = np.ascontiguousarray(np.asarray(sin, dtype=np.float32)[0].T)
    sc = np.float32(1.0 / math.sqrt(HD))
    if causal:
        md = _diag_mask_tiles()
    else:
        mT = np.ascontiguousarray(m2.T)

    in_maps = []
    for c in range(NCORES):
        im = {
            "hT": hT,
            "cosT": cosT,
            "sinT": sinT,
            "wqT": np.ascontiguousarray((wq[c * QH * HD : (c + 1) * QH * HD] * sc).T),
            "wkT": np.ascontiguousarray(wk[c * HD : (c + 1) * HD].T),
            "wvT": np.ascontiguousarray(wv[c * HD : (c + 1) * HD].T),
            "woT": np.ascontiguousarray(wo[:, c * QH * HD : (c + 1) * QH * HD].T),
        }
        if causal:
            im["maskd"] = md
        else:
            im["maskT"] = mT
        in_maps.append(im)
    return causal, in_maps


def kernel(hidden_states, attention_mask, cos, sin, wq, wk, wv, wo):
    causal, in_maps = make_in_maps(
        hidden_states, attention_mask, cos, sin, wq, wk, wv, wo
    )
    nc = _build(causal)
    res = run_bass_kernel_spmd(nc, in_maps, list(range(NCORES)))
    out = np.zeros((S, HID), np.float64)
    for c in range(NCORES):
        out += res.results[c]["y"].astype(np.float64)
    return out.reshape(B, S, HID).astype(np.float32)


# revision 6
# speedup vs baseline: 40.8575x; 40.8575x over previous
"""Trainium2 Bass kernel for MllamaTextSelfAttention (B=1, S=2048, HID=4096,
32 Q heads / 8 KV heads, HD=128, RoPE, causal mask, GQA).

Sharding: tensor-parallel over heads across 8 NeuronCores. Core c computes
Q heads [4c, 4c+4) and KV head c, plus the matching slice of the output
projection; the 8 partial outputs are summed on the host.

Device dataflow (per core, all matmuls in fp32r):
  - qT/kT/vT = W^T-stationary projections -> [d, s] layouts
  - RoPE applied in [d, s] layout (partition-shifted rotate-half)
  - V transposed to natural [s, d] via PE transpose
  - S^T[k, q] = kT-tile.T @ qT-stripe  (k-partition layout)
  - E = exp(S^T + mask)  (no max subtraction; scores are O(10), mask -1e9
    underflows exp to exactly 0)
  - den[1, q] = ones.T @ E  (partition reduction on PE)
  - O^T[d, q] = V-tile.T @ E  accumulated over k tiles
  - O^T *= broadcast(1/den)  (rank-1 ones x recip matmul for the
    partition-direction broadcast)
  - Y[s, hid] = O^T-tiles.T @ woT  streamed to DRAM
"""

import math
import os
import sys

for _p in (
    "/opt/trn_rl_repo",
    "/root/.axon_site",
    "/root/.axon_site/_ro/trn_rl_repo",
    "/root/.axon_site/_ro/pypackages",
):
    if os.path.isdir(_p) and _p not in sys.path:
        sys.path.append(_p)

import numpy as np
from contextlib import ExitStack

import concourse.bass as bass
import concourse.tile as tile
from concourse import mybir
from concourse.bass_utils import run_bass_kernel_spmd
from concourse.masks import make_identity

F32 = mybir.dt.float32
FR = mybir.dt.float32r
ACTF = mybir.ActivationFunctionType

B, S, HID = 1, 2048, 4096
NH, NKV, HD = 32, 8, 128
NCORES = 8
QH = NH // NCORES          # 4 q heads per core
SS = 512                   # sequence stripe (matmul free dim)
NQS = S // SS              # 4 stripes
NKT = S // 128             # 16 k tiles
KH = HID // 128            # 32 hidden-dim k tiles
NEG = -1e9


def _split_multi_waits(nc: bass.Bass):
    """Walrus in this container encodes at most ONE sync-wait command per
    instruction. Hoist extra waits onto injected same-engine NoOps placed
    immediately before the instruction; engines are in-order so the
    semantics are unchanged."""
    n = 0
    for fn in nc.m.functions:
        for bb in fn.blocks:
            out = []
            for inst in bb.instructions:
                si = inst.sync_info
                if si is not None and si.on_wait and len(si.on_wait) > 1:
                    waits = list(si.on_wait)
                    for w in waits[:-1]:
                        n += 1
                        nop = mybir.InstNoOp(name=f"I-swait-{n}", ins=[], outs=[])
                        nop.engine = inst.engine
                        nop.sync_info = mybir.SyncInfo(on_wait=[w], on_update=[])
                        out.append(nop)
                    si.on_wait = [waits[-1]]
                out.append(inst)
            bb.instructions[:] = out
    return nc


_BUILD_CACHE = {}


def _build(causal: bool, split_waits: bool = True, loop_n=None) -> bass.Bass:
    key = (causal, split_waits, loop_n)
    if key in _BUILD_CACHE:
        return _BUILD_CACHE[key]

    nc = bass.Bass()
    hT = nc.dram_tensor("hT", [HID, S], F32, kind="ExternalInput")
    wqT = nc.dram_tensor("wqT", [HID, QH * HD], F32, kind="ExternalInput")
    wkT = nc.dram_tensor("wkT", [HID, HD], F32, kind="ExternalInput")
    wvT = nc.dram_tensor("wvT", [HID, HD], F32, kind="ExternalInput")
    woT = nc.dram_tensor("woT", [QH * HD, HID], F32, kind="ExternalInput")
    cosT = nc.dram_tensor("cosT", [HD, S], F32, kind="ExternalInput")
    sinT = nc.dram_tensor("sinT", [HD, S], F32, kind="ExternalInput")
    if causal:
        maskd = nc.dram_tensor("maskd", [128, 4 * SS], F32, kind="ExternalInput")
    else:
        maskT = nc.dram_tensor("maskT", [S, S], F32, kind="ExternalInput")
    y = nc.dram_tensor("y", [S, HID], F32, kind="ExternalOutput")

    with tile.TileContext(nc) as tc, ExitStack() as ctx:
        if loop_n is not None:
            # device-side repeat loop for dispatch-amortized timing
            ctx.enter_context(tc.For_i(0, loop_n, 1))
        outer = ctx.enter_context(tc.tile_pool(name="outer", bufs=1))
        qT = outer.tile([128, QH * S], FR)      # [d, h*s]
        kT = outer.tile([128, S], FR)           # [d, s]
        vT = outer.tile([128, S], F32)          # [d, s]
        v_sb = outer.tile([128, S], FR)         # [s-within-tile, t*d]
        ot = outer.tile([128, QH * S], FR)      # [d, h*s] normalized O^T

        # ---------------- phase 1: QKV projections ----------------
        with (
            tc.tile_pool(name="wqkv", bufs=1) as wp,
            tc.tile_pool(name="hstream", bufs=3) as hp,
            tc.tile_pool(name="ps1", bufs=1, space="PSUM") as pp1,
        ):
            wq_sb = wp.tile([128, KH * QH * HD], FR)
            wk_sb = wp.tile([128, KH * HD], FR)
            wv_sb = wp.tile([128, KH * HD], FR)
            for k in range(KH):
                nc.sync.dma_start(
                    wq_sb[:, k * 512 : (k + 1) * 512],
                    wqT[k * 128 : (k + 1) * 128, :].bitcast(FR),
                )
                nc.sync.dma_start(
                    wk_sb[:, k * 128 : (k + 1) * 128],
                    wkT[k * 128 : (k + 1) * 128, :].bitcast(FR),
                )
                nc.sync.dma_start(
                    wv_sb[:, k * 128 : (k + 1) * 128],
                    wvT[k * 128 : (k + 1) * 128, :].bitcast(FR),
                )

            for n in range(NQS):
                psq = [
                    pp1.tile([128, SS], F32, name=f"psq{m}", tag=f"psq{m}")
                    for m in range(QH)
                ]
                psk = pp1.tile([128, SS], F32, tag="psk")
                psv = pp1.tile([128, SS], F32, tag="psv")
                for k in range(KH):
                    ht = hp.tile([128, SS], FR, tag="ht")
                    nc.sync.dma_start(
                        ht[:],
                        hT[k * 128 : (k + 1) * 128, n * SS : (n + 1) * SS].bitcast(FR),
                    )
                    st, sp = (k == 0), (k == KH - 1)
                    for m in range(QH):
                        nc.tensor.matmul(
                            psq[m][:],
                            wq_sb[:, k * 512 + m * 128 : k * 512 + (m + 1) * 128],
                            ht[:],
                            start=st,
                            stop=sp,
                        )
                    nc.tensor.matmul(
                        psk[:],
                        wk_sb[:, k * 128 : (k + 1) * 128],
                        ht[:],
                        start=st,
                        stop=sp,
                    )
                    nc.tensor.matmul(
                        psv[:],
                        wv_sb[:, k * 128 : (k + 1) * 128],
                        ht[:],
                        start=st,
                        stop=sp,
                    )
                for m in range(QH):
                    nc.scalar.copy(
                        qT[:, m * S + n * SS : m * S + (n + 1) * SS], psq[m][:]
                    )
                nc.scalar.copy(kT[:, n * SS : (n + 1) * SS], psk[:])
                nc.scalar.copy(vT[:, n * SS : (n + 1) * SS], psv[:])

        # ---------------- phase 1.5: RoPE + V transpose ----------------
        with (
            tc.tile_pool(name="rope", bufs=1) as rp,
            tc.tile_pool(name="pst", bufs=2, space="PSUM") as ppt,
        ):
            cos_sb = rp.tile([128, S], F32)
            sin_sb = rp.tile([128, S], F32)
            nc.sync.dma_start(cos_sb[:], cosT[:, :])
            nc.sync.dma_start(sin_sb[:], sinT[:, :])
            for i in range(QH + 1):
                src = qT[:, i * S : (i + 1) * S] if i < QH else kT[:, :]
                rot = rp.tile([128, S], F32, tag="rot")
                tmp = rp.tile([128, S], F32, tag="tmp")
                # rotate_half in [d, s]: rot[0:64] = -src[64:128]; rot[64:128] = src[0:64]
                nc.vector.tensor_scalar_mul(rot[0:64, :], src[64:128, :], -1.0)
                nc.vector.tensor_copy(rot[64:128, :], src[0:64, :])
                nc.vector.tensor_mul(tmp[:], src, cos_sb[:])
                nc.vector.tensor_mul(rot[:], rot[:], sin_sb[:])
                nc.vector.tensor_add(src, tmp[:], rot[:])

            id_sb = rp.tile([128, 128], F32)
            make_identity(nc, id_sb[:])
            for t in range(NKT):
                pst = ppt.tile([128, 128], F32, tag="pst")
                nc.tensor.transpose(pst[:], vT[:, t * 128 : (t + 1) * 128], id_sb[:])
                nc.scalar.copy(v_sb[:, t * 128 : (t + 1) * 128], pst[:])

        # ---------------- phase 2: attention ----------------
        with (
            tc.tile_pool(name="att", bufs=1) as ap_,
            tc.tile_pool(name="epool", bufs=2 if causal else 1) as ep,
            tc.tile_pool(name="mrowp", bufs=1) as mp,
            tc.tile_pool(name="ps2s", bufs=2, space="PSUM") as pp2s,
            tc.tile_pool(name="ps2a", bufs=1, space="PSUM") as pp2a,
        ):
            ones_f32 = ap_.tile([128, 128], F32)
            nc.vector.memset(ones_f32[:], 1.0)
            ones_sb = ap_.tile([128, 128], FR)
            nc.vector.tensor_copy(ones_sb[:], ones_f32[:])
            if causal:
                md_sb = ap_.tile([128, 4 * SS], F32)
                nc.sync.dma_start(md_sb[:], maskd[:, :])

            for qs in range(NQS):
                nkt = 4 * qs + 4 if causal else NKT
                if not causal:
                    mrow = mp.tile([128, NKT * SS], F32, tag="mrow")
                    for t in range(NKT):
                        nc.sync.dma_start(
                            mrow[:, t * SS : (t + 1) * SS],
                            maskT[t * 128 : (t + 1) * 128, qs * SS : (qs + 1) * SS],
                        )
                for h in range(QH):
                    e = ep.tile([128, NKT * SS], FR, tag="e")
                    qsl = qT[:, h * S + qs * SS : h * S + (qs + 1) * SS]
                    for t in range(nkt):
                        pss = pp2s.tile([128, SS], F32, tag="pss")
                        nc.tensor.matmul(
                            pss[:],
                            kT[:, t * 128 : (t + 1) * 128],
                            qsl,
                            start=True,
                            stop=True,
                        )
                        dst = e[:, t * SS : (t + 1) * SS]
                        if causal and t >= 4 * qs:
                            j = t - 4 * qs
                            nc.vector.tensor_add(
                                dst, pss[:], md_sb[:, j * SS : (j + 1) * SS]
                            )
                            nc.scalar.activation(dst, dst, ACTF.Exp)
                        elif not causal:
                            nc.vector.tensor_add(
                                dst, pss[:], mrow[:, t * SS : (t + 1) * SS]
                            )
                            nc.scalar.activation(dst, dst, ACTF.Exp)
                        else:
                            nc.scalar.activation(dst, pss[:], ACTF.Exp)

                    psd = pp2a.tile([1, SS], F32, tag="psd")
                    pso = pp2a.tile([128, SS], F32, tag="pso")
                    for t in range(nkt):
                        er = e[:, t * SS : (t + 1) * SS]
                        st, sp = (t == 0), (t == nkt - 1)
                        nc.tensor.matmul(
                            psd[:], ones_sb[:, 0:1], er, start=st, stop=sp
                        )
                        nc.tensor.matmul(
                            pso[:],
                            v_sb[:, t * 128 : (t + 1) * 128],
                            er,
                            start=st,
                            stop=sp,
                        )
                    den = ap_.tile([1, SS], F32, tag="den")
                    nc.scalar.copy(den[:], psd[:])
                    rec = ap_.tile([1, SS], FR, tag="rec")
                    with nc.allow_low_precision(reason="fp32r recip feeds matmul"):
                        nc.vector.reciprocal(rec[:], den[:])
                    psb = pp2a.tile([128, SS], F32, tag="psb")
                    nc.tensor.matmul(
                        psb[:],
                        ones_sb[0:1, 0:128],
                        rec[:],
                        start=True,
                        stop=True,
                    )
                    od = ot[:, h * S + qs * SS : h * S + (qs + 1) * SS]
                    nc.scalar.copy(od, pso[:])
                    nc.vector.tensor_mul(od, od, psb[:])

        # ---------------- phase 3: output projection ----------------
        with (
            tc.tile_pool(name="wop", bufs=1) as wop,
            tc.tile_pool(name="yout", bufs=3) as yp,
            tc.tile_pool(name="ps3", bufs=2, space="PSUM") as pp3,
        ):
            wo_sb = wop.tile([128, QH * HID], FR)
            for hh in range(QH):
                nc.sync.dma_start(
                    wo_sb[:, hh * HID : (hh + 1) * HID],
                    woT[hh * 128 : (hh + 1) * 128, :].bitcast(FR),
                )
            for st in range(NKT):
                for nn in range(HID // SS):
                    psy = pp3.tile([128, SS], F32, tag="psy")
                    for hh in range(QH):
                        nc.tensor.matmul(
                            psy[:],
                            ot[:, hh * S + st * 128 : hh * S + (st + 1) * 128],
                            wo_sb[:, hh * HID + nn * SS : hh * HID + (nn + 1) * SS],
                            start=(hh == 0),
                            stop=(hh == QH - 1),
                        )
                    yt = yp.tile([128, SS], F32, tag="yt")
                    nc.scalar.copy(yt[:], psy[:])
                    nc.sync.dma_start(
                        y[st * 128 : (st + 1) * 128, nn * SS : (nn + 1) * SS], yt[:]
                    )

    if split_waits:
        _split_multi_waits(nc)
    _BUILD_CACHE[key] = nc
    return nc


def _causal_mask_ref() -> np.ndarray:
    return np.triu(np.full((S, S), NEG, np.float32), k=1)


def _diag_mask_tiles() -> np.ndarray:
    p = np.arange(128, dtype=np.int64)[:, None]
    f = np.arange(SS, dtype=np.int64)[None, :]
    cols = [
        np.where(128 * j + p > f, np.float32(NEG), np.float32(0.0)) for j in range(4)
    ]
    return np.ascontiguousarray(np.concatenate(cols, axis=1).astype(np.float32))


def make_in_maps(hidden_states, attention_mask, cos, sin, wq, wk, wv, wo):
    """Host-side sharding/preprocessing. Returns (causal, in_maps)."""
    h = np.ascontiguousarray(np.asarray(hidden_states, dtype=np.float32)[0])
    m2 = np.ascontiguousarray(np.asarray(attention_mask, dtype=np.float32)[0, 0])
    wq = np.asarray(wq, dtype=np.float32)
    wk = np.asarray(wk, dtype=np.float32)
    wv = np.asarray(wv, dtype=np.float32)
    wo = np.asarray(wo, dtype=np.float32)

    causal = bool(np.array_equal(m2, _causal_mask_ref()))
    hT = np.ascontiguousarray(h.T)
    cosT = np.ascontiguousarray(np.asarray(cos, dtype=np.float32)[0].T)
    sinT = np.ascontiguousarray(np.asarray(sin, dtype=np.float32)[0].T)
    sc = np.float32(1.0 / math.sqrt(HD))
    if causal:
        md = _diag_mask_tiles()
    else:
        mT = np.ascontiguousarray(m2.T)

    in_maps = []
    for c in range(NCORES):
        im = {
            "hT": hT,
            "cosT": cosT,
            "sinT": sinT,
            "wqT": np.ascontiguousarray((wq[c * QH * HD : (c + 1) * QH * HD] * sc).T),
            "wkT": np.ascontiguousarray(wk[c * HD : (c + 1) * HD].T),
            "wvT": np.ascontiguousarray(wv[c * HD : (c + 1) * HD].T),
            "woT": np.ascontiguousarray(wo[:, c * QH * HD : (c + 1) * QH * HD].T),
        }
        if causal:
            im["maskd"] = md
        else:
            im["maskT"] = mT
        in_maps.append(im)
    return causal, in_maps


def kernel(hidden_states, attention_mask, cos, sin, wq, wk, wv, wo):
    causal, in_maps = make_in_maps(
        hidden_states, attention_mask, cos, sin, wq, wk, wv, wo
    )
    nc = _build(causal)
    res = run_bass_kernel_spmd(nc, in_maps, list(range(NCORES)))
    out = np.zeros((S, HID), np.float64)
    for c in range(NCORES):
        out += res.results[c]["y"].astype(np.float64)
    return out.reshape(B, S, HID).astype(np.float32)


# revision 8
# speedup vs baseline: 47.5961x; 1.1649x over previous
"""Trainium2 Bass kernel for MllamaTextSelfAttention (B=1, S=2048, HID=4096,
32 Q heads / 8 KV heads, HD=128, RoPE, causal mask, GQA).

Sharding: tensor-parallel over heads across 8 NeuronCores. Core c computes
Q heads [4c, 4c+4) and KV head c, plus the matching slice of the output
projection; the 8 partial outputs are summed on the host.

Device dataflow (per core, all matmuls in fp32r):
  - qT/kT/vT = W^T-stationary projections -> [d, s] layouts
  - RoPE applied in [d, s] layout (partition-shifted rotate-half)
  - V transposed to natural [s, d] via PE transpose
  - S^T[k, q] = kT-tile.T @ qT-stripe  (k-partition layout)
  - E = exp(S^T + mask)  (no max subtraction; scores are O(10), mask -1e9
    underflows exp to exactly 0)
  - den[1, q] = ones.T @ E  (partition reduction on PE)
  - O^T[d, q] = V-tile.T @ E  accumulated over k tiles
  - O^T *= broadcast(1/den)  (rank-1 ones x recip matmul for the
    partition-direction broadcast)
  - Y[s, hid] = O^T-tiles.T @ woT  streamed to DRAM
"""

import math
import os
import sys

for _p in (
    "/opt/trn_rl_repo",
    "/root/.axon_site",
    "/root/.axon_site/_ro/trn_rl_repo",
    "/root/.axon_site/_ro/pypackages",
):
    if os.path.isdir(_p) and _p not in sys.path:
        sys.path.append(_p)

import numpy as np
from contextlib import ExitStack

import concourse.bass as bass
import concourse.tile as tile
from concourse import mybir
from concourse.bass_utils import run_bass_kernel_spmd
from concourse.masks import make_identity

F32 = mybir.dt.float32
FR = mybir.dt.float32r
ACTF = mybir.ActivationFunctionType

B, S, HID = 1, 2048, 4096
NH, NKV, HD = 32, 8, 128
NCORES = 8
QH = NH // NCORES          # 4 q heads per core
SS = 512                   # sequence stripe (matmul free dim)
NQS = S // SS              # 4 stripes
NKT = S // 128             # 16 k tiles
KH = HID // 128            # 32 hidden-dim k tiles
NEG = -1e9


def _split_multi_waits(nc: bass.Bass):
    """Walrus in this container encodes at most ONE sync-wait command per
    instruction. Hoist extra waits onto injected same-engine NoOps placed
    immediately before the instruction; engines are in-order so the
    semantics are unchanged."""
    n = 0
    for fn in nc.m.functions:
        for bb in fn.blocks:
            out = []
            for inst in bb.instructions:
                si = inst.sync_info
                if si is not None and si.on_wait and len(si.on_wait) > 1:
                    waits = list(si.on_wait)
                    for w in waits[:-1]:
                        n += 1
                        nop = mybir.InstNoOp(name=f"I-swait-{n}", ins=[], outs=[])
                        nop.engine = inst.engine
                        nop.sync_info = mybir.SyncInfo(on_wait=[w], on_update=[])
                        out.append(nop)
                    si.on_wait = [waits[-1]]
                out.append(inst)
            bb.instructions[:] = out
    return nc


_BUILD_CACHE = {}


def _build(causal: bool, split_waits: bool = True, loop_n=None) -> bass.Bass:
    key = (causal, split_waits, loop_n)
    if key in _BUILD_CACHE:
        return _BUILD_CACHE[key]

    nc = bass.Bass()
    hT = nc.dram_tensor("hT", [HID, S], F32, kind="ExternalInput")
    wqT = nc.dram_tensor("wqT", [HID, QH * HD], F32, kind="ExternalInput")
    wkT = nc.dram_tensor("wkT", [HID, HD], F32, kind="ExternalInput")
    wvT = nc.dram_tensor("wvT", [HID, HD], F32, kind="ExternalInput")
    woT = nc.dram_tensor("woT", [QH * HD, HID], F32, kind="ExternalInput")
    cosT = nc.dram_tensor("cosT", [HD, S], F32, kind="ExternalInput")
    sinT = nc.dram_tensor("sinT", [HD, S], F32, kind="ExternalInput")
    if causal:
        maskd = nc.dram_tensor("maskd", [128, 4 * SS], F32, kind="ExternalInput")
    else:
        maskT = nc.dram_tensor("maskT", [S, S], F32, kind="ExternalInput")
    y = nc.dram_tensor("y", [S, HID], F32, kind="ExternalOutput")

    with tile.TileContext(nc) as tc, ExitStack() as ctx:
        if loop_n is not None:
            # device-side repeat loop for dispatch-amortized timing
            ctx.enter_context(tc.For_i(0, loop_n, 1))
        # SWDGE (gpsimd) DMA inside a For_i fails this walrus' codegen, so
        # the timing variant issues everything from SP instead
        gp = nc.sync if loop_n is not None else nc.gpsimd
        outer = ctx.enter_context(tc.tile_pool(name="outer", bufs=1))
        qT = outer.tile([128, QH * S], FR)      # [d, h*s]
        kT = outer.tile([128, S], FR)           # [d, s]
        vT = outer.tile([128, S], F32)          # [d, s]
        v_sb = outer.tile([128, S], FR)         # [s-within-tile, t*d]
        ot = outer.tile([128, QH * S], FR)      # [d, h*s] normalized O^T

        # ---------------- phase 1: QKV projections ----------------
        with (
            tc.tile_pool(name="wqkv", bufs=1) as wp,
            tc.tile_pool(name="hstream", bufs=3) as hp,
            tc.tile_pool(name="ps1", bufs=1, space="PSUM") as pp1,
        ):
            wq_sb = wp.tile([128, KH * QH * HD], FR)
            wk_sb = wp.tile([128, KH * HD], FR)
            wv_sb = wp.tile([128, KH * HD], FR)
            for k in range(KH):
                gp.dma_start(
                    wq_sb[:, k * 512 : (k + 1) * 512],
                    wqT[k * 128 : (k + 1) * 128, :].bitcast(FR),
                )
                gp.dma_start(
                    wk_sb[:, k * 128 : (k + 1) * 128],
                    wkT[k * 128 : (k + 1) * 128, :].bitcast(FR),
                )
                gp.dma_start(
                    wv_sb[:, k * 128 : (k + 1) * 128],
                    wvT[k * 128 : (k + 1) * 128, :].bitcast(FR),
                )

            for n in range(NQS):
                psq = [
                    pp1.tile([128, SS], F32, name=f"psq{m}", tag=f"psq{m}")
                    for m in range(QH)
                ]
                psk = pp1.tile([128, SS], F32, tag="psk")
                psv = pp1.tile([128, SS], F32, tag="psv")
                for k in range(KH):
                    ht = hp.tile([128, SS], FR, tag="ht")
                    dma_eng = nc.sync if (k % 2 == 0) else gp
                    dma_eng.dma_start(
                        ht[:],
                        hT[k * 128 : (k + 1) * 128, n * SS : (n + 1) * SS].bitcast(FR),
                    )
                    st, sp = (k == 0), (k == KH - 1)
                    for m in range(QH):
                        nc.tensor.matmul(
                            psq[m][:],
                            wq_sb[:, k * 512 + m * 128 : k * 512 + (m + 1) * 128],
                            ht[:],
                            start=st,
                            stop=sp,
                        )
                    nc.tensor.matmul(
                        psk[:],
                        wk_sb[:, k * 128 : (k + 1) * 128],
                        ht[:],
                        start=st,
                        stop=sp,
                    )
                    nc.tensor.matmul(
                        psv[:],
                        wv_sb[:, k * 128 : (k + 1) * 128],
                        ht[:],
                        start=st,
                        stop=sp,
                    )
                for m in range(QH):
                    nc.vector.tensor_copy(
                        qT[:, m * S + n * SS : m * S + (n + 1) * SS], psq[m][:]
                    )
                nc.vector.tensor_copy(kT[:, n * SS : (n + 1) * SS], psk[:])
                nc.vector.tensor_copy(vT[:, n * SS : (n + 1) * SS], psv[:])

        # ---------------- phase 1.5: RoPE + V transpose ----------------
        with (
            tc.tile_pool(name="rope", bufs=1) as rp,
            tc.tile_pool(name="pst", bufs=2, space="PSUM") as ppt,
        ):
            cos_sb = rp.tile([128, S], F32)
            sin_sb = rp.tile([128, S], F32)
            nc.sync.dma_start(cos_sb[:], cosT[:, :])
            nc.sync.dma_start(sin_sb[:], sinT[:, :])
            for i in range(QH + 1):
                src = qT[:, i * S : (i + 1) * S] if i < QH else kT[:, :]
                rot = rp.tile([128, S], F32, tag="rot")
                tmp = rp.tile([128, S], F32, tag="tmp")
                # rotate_half in [d, s]: rot[0:64] = -src[64:128]; rot[64:128] = src[0:64]
                nc.vector.tensor_scalar_mul(rot[0:64, :], src[64:128, :], -1.0)
                nc.vector.tensor_copy(rot[64:128, :], src[0:64, :])
                nc.vector.tensor_mul(tmp[:], src, cos_sb[:])
                nc.vector.tensor_mul(rot[:], rot[:], sin_sb[:])
                nc.vector.tensor_add(src, tmp[:], rot[:])

            id_sb = rp.tile([128, 128], F32)
            make_identity(nc, id_sb[:])
            for t in range(NKT):
                pst = ppt.tile([128, 128], F32, tag="pst")
                nc.tensor.transpose(pst[:], vT[:, t * 128 : (t + 1) * 128], id_sb[:])
                nc.vector.tensor_copy(v_sb[:, t * 128 : (t + 1) * 128], pst[:])

        # ---------------- phase 2: attention ----------------
        with (
            tc.tile_pool(name="att", bufs=1) as ap_,
            tc.tile_pool(name="epool", bufs=2 if causal else 1) as ep,
            tc.tile_pool(name="mrowp", bufs=1) as mp,
            tc.tile_pool(name="ps2s", bufs=2, space="PSUM") as pp2s,
            tc.tile_pool(name="ps2a", bufs=1, space="PSUM") as pp2a,
        ):
            ones_f32 = ap_.tile([128, 128], F32)
            nc.vector.memset(ones_f32[:], 1.0)
            ones_sb = ap_.tile([128, 128], FR)
            nc.vector.tensor_copy(ones_sb[:], ones_f32[:])
            if causal:
                md_sb = ap_.tile([128, 4 * SS], F32)
                nc.sync.dma_start(md_sb[:], maskd[:, :])

            for qs in range(NQS):
                nkt = 4 * qs + 4 if causal else NKT
                if not causal:
                    mrow = mp.tile([128, NKT * SS], F32, tag="mrow")
                    for t in range(NKT):
                        nc.sync.dma_start(
                            mrow[:, t * SS : (t + 1) * SS],
                            maskT[t * 128 : (t + 1) * 128, qs * SS : (qs + 1) * SS],
                        )
                for h in range(QH):
                    e = ep.tile([128, NKT * SS], FR, tag="e")
                    qsl = qT[:, h * S + qs * SS : h * S + (qs + 1) * SS]
                    for t in range(nkt):
                        pss = pp2s.tile([128, SS], F32, tag="pss")
                        nc.tensor.matmul(
                            pss[:],
                            kT[:, t * 128 : (t + 1) * 128],
                            qsl,
                            start=True,
                            stop=True,
                        )
                        dst = e[:, t * SS : (t + 1) * SS]
                        if causal and t >= 4 * qs:
                            j = t - 4 * qs
                            nc.vector.tensor_add(
                                dst, pss[:], md_sb[:, j * SS : (j + 1) * SS]
                            )
                            nc.scalar.activation(dst, dst, ACTF.Exp)
                        elif not causal:
                            nc.vector.tensor_add(
                                dst, pss[:], mrow[:, t * SS : (t + 1) * SS]
                            )
                            nc.scalar.activation(dst, dst, ACTF.Exp)
                        else:
                            nc.scalar.activation(dst, pss[:], ACTF.Exp)

                    psd = pp2a.tile([1, SS], F32, tag="psd")
                    pso = pp2a.tile([128, SS], F32, tag="pso")
                    for t in range(nkt):
                        er = e[:, t * SS : (t + 1) * SS]
                        st, sp = (t == 0), (t == nkt - 1)
                        nc.tensor.matmul(
                            psd[:], ones_sb[:, 0:1], er, start=st, stop=sp
                        )
                        nc.tensor.matmul(
                            pso[:],
                            v_sb[:, t * 128 : (t + 1) * 128],
                            er,
                            start=st,
                            stop=sp,
                        )
                    den = ap_.tile([1, SS], F32, tag="den")
                    nc.vector.tensor_copy(den[:], psd[:])
                    rec = ap_.tile([1, SS], FR, tag="rec")
                    with nc.allow_low_precision(reason="fp32r recip feeds matmul"):
                        nc.vector.reciprocal(rec[:], den[:])
                    psb = pp2a.tile([128, SS], F32, tag="psb")
                    nc.tensor.matmul(
                        psb[:],
                        ones_sb[0:1, 0:128],
                        rec[:],
                        start=True,
                        stop=True,
                    )
                    od = ot[:, h * S + qs * SS : h * S + (qs + 1) * SS]
                    nc.vector.tensor_copy(od, pso[:])
                    nc.vector.tensor_mul(od, od, psb[:])

        # ---------------- phase 3: output projection ----------------
        with (
            tc.tile_pool(name="wop", bufs=1) as wop,
            tc.tile_pool(name="yout", bufs=2) as yp,
            tc.tile_pool(name="ps3", bufs=2, space="PSUM") as pp3,
        ):
            wo_sb = wop.tile([128, QH * HID], FR)
            for hh in range(QH):
                gp.dma_start(
                    wo_sb[:, hh * HID : (hh + 1) * HID],
                    woT[hh * 128 : (hh + 1) * 128, :].bitcast(FR),
                )
            for st in range(NKT):
                yt = yp.tile([128, HID], F32, tag="yt")
                for nn in range(HID // SS):
                    psy = pp3.tile([128, SS], F32, tag="psy")
                    for hh in range(QH):
                        nc.tensor.matmul(
                            psy[:],
                            ot[:, hh * S + st * 128 : hh * S + (st + 1) * 128],
                            wo_sb[:, hh * HID + nn * SS : hh * HID + (nn + 1) * SS],
                            start=(hh == 0),
                            stop=(hh == QH - 1),
                        )
                    nc.vector.tensor_copy(yt[:, nn * SS : (nn + 1) * SS], psy[:])
                eng = nc.sync if (st % 2 == 0) else gp
                eng.dma_start(y[st * 128 : (st + 1) * 128, :], yt[:])

    if split_waits:
        _split_multi_waits(nc)
    _BUILD_CACHE[key] = nc
    return nc


def _causal_mask_ref() -> np.ndarray:
    return np.triu(np.full((S, S), NEG, np.float32), k=1)


def _diag_mask_tiles() -> np.ndarray:
    p = np.arange(128, dtype=np.int64)[:, None]
    f = np.arange(SS, dtype=np.int64)[None, :]
    cols = [
        np.where(128 * j + p > f, np.float32(NEG), np.float32(0.0)) for j in range(4)
    ]
    return np.ascontiguousarray(np.concatenate(cols, axis=1).astype(np.float32))


def make_in_maps(hidden_states, attention_mask, cos, sin, wq, wk, wv, wo):
    """Host-side sharding/preprocessing. Returns (causal, in_maps)."""
    h = np.ascontiguousarray(np.asarray(hidden_states, dtype=np.float32)[0])
    m2 = np.ascontiguousarray(np.asarray(attention_mask, dtype=np.float32)[0, 0])
    wq = np.asarray(wq, dtype=np.float32)
    wk = np.asarray(wk, dtype=np.float32)
    wv = np.asarray(wv, dtype=np.float32)
    wo = np.asarray(wo, dtype=np.float32)

    causal = bool(np.array_equal(m2, _causal_mask_ref()))
    hT = np.ascontiguousarray(h.T)
    cosT = np.ascontiguousarray(np.asarray(cos, dtype=np.float32)[0].T)
    sinT = np.ascontiguousarray(np.asarray(sin, dtype=np.float32)[0].T)
    sc = np.float32(1.0 / math.sqrt(HD))
    if causal:
        md = _diag_mask_tiles()
    else:
        mT = np.ascontiguousarray(m2.T)

    in_maps = []
    for c in range(NCORES):
        im = {
            "hT": hT,
            "cosT": cosT,
            "sinT": sinT,
            "wqT": np.ascontiguousarray((wq[c * QH * HD : (c + 1) * QH * HD] * sc).T),
            "wkT": np.ascontiguousarray(wk[c * HD : (c + 1) * HD].T),
            "wvT": np.ascontiguousarray(wv[c * HD : (c + 1) * HD].T),
            "woT": np.ascontiguousarray(wo[:, c * QH * HD : (c + 1) * QH * HD].T),
        }
        if causal:
            im["maskd"] = md
        else:
            im["maskT"] = mT
        in_maps.append(im)
    return causal, in_maps


def kernel(hidden_states, attention_mask, cos, sin, wq, wk, wv, wo):
    causal, in_maps = make_in_maps(
        hidden_states, attention_mask, cos, sin, wq, wk, wv, wo
    )
    nc = _build(causal)
    res = run_bass_kernel_spmd(nc, in_maps, list(range(NCORES)))
    out = np.zeros((S, HID), np.float64)
    for c in range(NCORES):
        out += res.results[c]["y"].astype(np.float64)
    return out.reshape(B, S, HID).astype(np.float32)


# revision 9
# speedup vs baseline: 49.5000x; 1.0400x over previous
"""Trainium2 Bass kernel for MllamaTextSelfAttention (B=1, S=2048, HID=4096,
32 Q heads / 8 KV heads, HD=128, RoPE, causal mask, GQA).

Sharding: tensor-parallel over heads across 8 NeuronCores. Core c computes
Q heads [4c, 4c+4) and KV head c, plus the matching slice of the output
projection; the 8 partial outputs are summed on the host.

Device dataflow (per core, all matmuls in fp32r):
  - qT/kT/vT = W^T-stationary projections -> [d, s] layouts
  - RoPE applied in [d, s] layout (partition-shifted rotate-half)
  - V transposed to natural [s, d] via PE transpose
  - S^T[k, q] = kT-tile.T @ qT-stripe  (k-partition layout)
  - E = exp(S^T + mask)  (no max subtraction; scores are O(10), mask -1e9
    underflows exp to exactly 0)
  - den[1, q] = ones.T @ E  (partition reduction on PE)
  - O^T[d, q] = V-tile.T @ E  accumulated over k tiles
  - O^T *= broadcast(1/den)  (rank-1 ones x recip matmul for the
    partition-direction broadcast)
  - Y[s, hid] = O^T-tiles.T @ woT  streamed to DRAM
"""

import math
import os
import sys

for _p in (
    "/opt/trn_rl_repo",
    "/root/.axon_site",
    "/root/.axon_site/_ro/trn_rl_repo",
    "/root/.axon_site/_ro/pypackages",
):
    if os.path.isdir(_p) and _p not in sys.path:
        sys.path.append(_p)

import numpy as np
from contextlib import ExitStack

import concourse.bass as bass
import concourse.tile as tile
from concourse import mybir
from concourse.bass_utils import run_bass_kernel_spmd
from concourse.masks import make_identity

F32 = mybir.dt.float32
FR = mybir.dt.float32r
ACTF = mybir.ActivationFunctionType

B, S, HID = 1, 2048, 4096
NH, NKV, HD = 32, 8, 128
NCORES = 8
QH = NH // NCORES          # 4 q heads per core
SS = 512                   # sequence stripe (matmul free dim)
NQS = S // SS              # 4 stripes
NKT = S // 128             # 16 k tiles
KH = HID // 128            # 32 hidden-dim k tiles
NEG = -1e9


def _split_multi_waits(nc: bass.Bass):
    """Walrus in this container encodes at most ONE sync-wait command per
    instruction. Hoist extra waits onto injected same-engine NoOps placed
    immediately before the instruction; engines are in-order so the
    semantics are unchanged."""
    n = 0
    for fn in nc.m.functions:
        for bb in fn.blocks:
            out = []
            for inst in bb.instructions:
                si = inst.sync_info
                if si is not None and si.on_wait and len(si.on_wait) > 1:
                    waits = list(si.on_wait)
                    for w in waits[:-1]:
                        n += 1
                        nop = mybir.InstNoOp(name=f"I-swait-{n}", ins=[], outs=[])
                        nop.engine = inst.engine
                        nop.sync_info = mybir.SyncInfo(on_wait=[w], on_update=[])
                        out.append(nop)
                    si.on_wait = [waits[-1]]
                out.append(inst)
            bb.instructions[:] = out
    return nc


_BUILD_CACHE = {}


def _build(causal: bool, split_waits: bool = True, loop_n=None) -> bass.Bass:
    key = (causal, split_waits, loop_n)
    if key in _BUILD_CACHE:
        return _BUILD_CACHE[key]

    nc = bass.Bass()
    hT = nc.dram_tensor("hT", [HID, S], F32, kind="ExternalInput")
    wqT = nc.dram_tensor("wqT", [HID, QH * HD], F32, kind="ExternalInput")
    wkT = nc.dram_tensor("wkT", [HID, HD], F32, kind="ExternalInput")
    wvT = nc.dram_tensor("wvT", [HID, HD], F32, kind="ExternalInput")
    woT = nc.dram_tensor("woT", [QH * HD, HID], F32, kind="ExternalInput")
    cosT = nc.dram_tensor("cosT", [HD, S], F32, kind="ExternalInput")
    sinT = nc.dram_tensor("sinT", [HD, S], F32, kind="ExternalInput")
    if causal:
        maskd = nc.dram_tensor("maskd", [128, 4 * SS], F32, kind="ExternalInput")
    else:
        maskT = nc.dram_tensor("maskT", [S, S], F32, kind="ExternalInput")
    y = nc.dram_tensor("y", [S, HID], F32, kind="ExternalOutput")

    with tile.TileContext(nc) as tc, ExitStack() as ctx:
        if loop_n is not None:
            # device-side repeat loop for dispatch-amortized timing
            ctx.enter_context(tc.For_i(0, loop_n, 1))
        # SWDGE (gpsimd) DMA inside a For_i fails this walrus' codegen, so
        # the timing variant issues everything from SP instead
        gp = nc.sync if loop_n is not None else nc.gpsimd
        outer = ctx.enter_context(tc.tile_pool(name="outer", bufs=1))
        qT = outer.tile([128, QH * S], FR)      # [d, h*s]
        kT = outer.tile([128, S], FR)           # [d, s]
        vT = outer.tile([128, S], F32)          # [d, s]
        v_sb = outer.tile([128, S], FR)         # [s-within-tile, t*d]
        ot = outer.tile([128, QH * S], FR)      # [d, h*s] normalized O^T

        # ---------------- phase 1: QKV projections ----------------
        with (
            tc.tile_pool(name="wqkv", bufs=1) as wp,
            tc.tile_pool(name="hstream", bufs=3) as hp,
            tc.tile_pool(name="ps1", bufs=1, space="PSUM") as pp1,
        ):
            wq_c = [wp.tile([128, 512], FR, name=f"wqc{k}", tag=f"wqc{k}") for k in range(KH)]
            wk_c = [wp.tile([128, HD], FR, name=f"wkc{k}", tag=f"wkc{k}") for k in range(KH)]
            wv_c = [wp.tile([128, HD], FR, name=f"wvc{k}", tag=f"wvc{k}") for k in range(KH)]
            for k in range(KH):
                gp.dma_start(wq_c[k][:], wqT[k * 128 : (k + 1) * 128, :].bitcast(FR))
                gp.dma_start(wk_c[k][:], wkT[k * 128 : (k + 1) * 128, :].bitcast(FR))
                gp.dma_start(wv_c[k][:], wvT[k * 128 : (k + 1) * 128, :].bitcast(FR))

            for n in range(NQS):
                psq = [
                    pp1.tile([128, SS], F32, name=f"psq{m}", tag=f"psq{m}")
                    for m in range(QH)
                ]
                psk = pp1.tile([128, SS], F32, tag="psk")
                psv = pp1.tile([128, SS], F32, tag="psv")
                for k in range(KH):
                    ht = hp.tile([128, SS], FR, tag="ht")
                    dma_eng = nc.sync if (k % 2 == 0) else gp
                    dma_eng.dma_start(
                        ht[:],
                        hT[k * 128 : (k + 1) * 128, n * SS : (n + 1) * SS].bitcast(FR),
                    )
                    st, sp = (k == 0), (k == KH - 1)
                    for m in range(QH):
                        nc.tensor.matmul(
                            psq[m][:],
                            wq_c[k][:, m * 128 : (m + 1) * 128],
                            ht[:],
                            start=st,
                            stop=sp,
                        )
                    nc.tensor.matmul(psk[:], wk_c[k][:], ht[:], start=st, stop=sp)
                    nc.tensor.matmul(psv[:], wv_c[k][:], ht[:], start=st, stop=sp)
                for m in range(QH):
                    nc.scalar.copy(
                        qT[:, m * S + n * SS : m * S + (n + 1) * SS], psq[m][:]
                    )
                nc.scalar.copy(kT[:, n * SS : (n + 1) * SS], psk[:])
                nc.scalar.copy(vT[:, n * SS : (n + 1) * SS], psv[:])

        # ---------------- phase 1.5: RoPE + V transpose ----------------
        with (
            tc.tile_pool(name="rope", bufs=1) as rp,
            tc.tile_pool(name="pst", bufs=2, space="PSUM") as ppt,
        ):
            cos_sb = rp.tile([128, S], F32)
            sin_sb = rp.tile([128, S], F32)
            nc.sync.dma_start(cos_sb[:], cosT[:, :])
            nc.sync.dma_start(sin_sb[:], sinT[:, :])
            for i in range(QH + 1):
                src = qT[:, i * S : (i + 1) * S] if i < QH else kT[:, :]
                rot = rp.tile([128, S], F32, tag="rot")
                tmp = rp.tile([128, S], F32, tag="tmp")
                # rotate_half in [d, s]: rot[0:64] = -src[64:128]; rot[64:128] = src[0:64]
                nc.vector.tensor_scalar_mul(rot[0:64, :], src[64:128, :], -1.0)
                nc.vector.tensor_copy(rot[64:128, :], src[0:64, :])
                nc.vector.tensor_mul(tmp[:], src, cos_sb[:])
                nc.vector.tensor_mul(rot[:], rot[:], sin_sb[:])
                nc.vector.tensor_add(src, tmp[:], rot[:])

            id_sb = rp.tile([128, 128], F32)
            make_identity(nc, id_sb[:])
            for t in range(NKT):
                pst = ppt.tile([128, 128], F32, tag="pst")
                nc.tensor.transpose(pst[:], vT[:, t * 128 : (t + 1) * 128], id_sb[:])
                nc.scalar.copy(v_sb[:, t * 128 : (t + 1) * 128], pst[:])

        # ---------------- phase 2: attention ----------------
        with (
            tc.tile_pool(name="att", bufs=1) as ap_,
            tc.tile_pool(name="epool", bufs=2 if causal else 1) as ep,
            tc.tile_pool(name="mrowp", bufs=1) as mp,
            tc.tile_pool(name="ps2s", bufs=4, space="PSUM") as pp2s,
            tc.tile_pool(name="ps2a", bufs=1, space="PSUM") as pp2a,
        ):
            ones_f32 = ap_.tile([128, 128], F32)
            nc.vector.memset(ones_f32[:], 1.0)
            ones_sb = ap_.tile([128, 128], FR)
            nc.vector.tensor_copy(ones_sb[:], ones_f32[:])
            if causal:
                md_sb = ap_.tile([128, 4 * SS], F32)
                nc.sync.dma_start(md_sb[:], maskd[:, :])

            for qs in range(NQS):
                nkt = 4 * qs + 4 if causal else NKT
                if not causal:
                    mrow = mp.tile([128, NKT * SS], F32, tag="mrow")
                    for t in range(NKT):
                        nc.sync.dma_start(
                            mrow[:, t * SS : (t + 1) * SS],
                            maskT[t * 128 : (t + 1) * 128, qs * SS : (qs + 1) * SS],
                        )
                for h in range(QH):
                    e = ep.tile([128, NKT * SS], FR, tag="e")
                    qsl = qT[:, h * S + qs * SS : h * S + (qs + 1) * SS]
                    for t in range(nkt):
                        pss = pp2s.tile([128, SS], F32, tag="pss")
                        nc.tensor.matmul(
                            pss[:],
                            kT[:, t * 128 : (t + 1) * 128],
                            qsl,
                            start=True,
                            stop=True,
                        )
                        dst = e[:, t * SS : (t + 1) * SS]
                        if causal and t >= 4 * qs:
                            j = t - 4 * qs
                            nc.vector.tensor_add(
                                dst, pss[:], md_sb[:, j * SS : (j + 1) * SS]
                            )
                            nc.scalar.activation(dst, dst, ACTF.Exp)
                        elif not causal:
                            nc.vector.tensor_add(
                                dst, pss[:], mrow[:, t * SS : (t + 1) * SS]
                            )
                            nc.scalar.activation(dst, dst, ACTF.Exp)
                        else:
                            nc.scalar.activation(dst, pss[:], ACTF.Exp)

                    psd = pp2a.tile([1, SS], F32, tag="psd")
                    pso = pp2a.tile([128, SS], F32, tag="pso")
                    for t in range(nkt):
                        er = e[:, t * SS : (t + 1) * SS]
                        st, sp = (t == 0), (t == nkt - 1)
                        nc.tensor.matmul(
                            psd[:], ones_sb[:, 0:1], er, start=st, stop=sp
                        )
                        nc.tensor.matmul(
                            pso[:],
                            v_sb[:, t * 128 : (t + 1) * 128],
                            er,
                            start=st,
                            stop=sp,
                        )
                    den = ap_.tile([1, SS], F32, tag="den")
                    nc.vector.tensor_copy(den[:], psd[:])
                    rec = ap_.tile([1, SS], FR, tag="rec")
                    with nc.allow_low_precision(reason="fp32r recip feeds matmul"):
                        nc.vector.reciprocal(rec[:], den[:])
                    psb = pp2a.tile([128, SS], F32, tag="psb")
                    nc.tensor.matmul(
                        psb[:],
                        ones_sb[0:1, 0:128],
                        rec[:],
                        start=True,
                        stop=True,
                    )
                    od = ot[:, h * S + qs * SS : h * S + (qs + 1) * SS]
                    nc.vector.tensor_copy(od, pso[:])
                    nc.vector.tensor_mul(od, od, psb[:])

        # ---------------- phase 3: output projection ----------------
        with (
            tc.tile_pool(name="wop", bufs=1) as wop,
            tc.tile_pool(name="yout", bufs=2) as yp,
            tc.tile_pool(name="ps3", bufs=2, space="PSUM") as pp3,
        ):
            wo_sb = wop.tile([128, QH * HID], FR)
            for hh in range(QH):
                gp.dma_start(
                    wo_sb[:, hh * HID : (hh + 1) * HID],
                    woT[hh * 128 : (hh + 1) * 128, :].bitcast(FR),
                )
            for st in range(NKT):
                yt = yp.tile([128, HID], F32, tag="yt")
                for nn in range(HID // SS):
                    psy = pp3.tile([128, SS], F32, tag="psy")
                    for hh in range(QH):
                        nc.tensor.matmul(
                            psy[:],
                            ot[:, hh * S + st * 128 : hh * S + (st + 1) * 128],
                            wo_sb[:, hh * HID + nn * SS : hh * HID + (nn + 1) * SS],
                            start=(hh == 0),
                            stop=(hh == QH - 1),
                        )
                    nc.scalar.copy(yt[:, nn * SS : (nn + 1) * SS], psy[:])
                eng = nc.sync if (st % 2 == 0) else gp
                eng.dma_start(y[st * 128 : (st + 1) * 128, :], yt[:])

    if split_waits:
        _split_multi_waits(nc)
    _BUILD_CACHE[key] = nc
    return nc


def _causal_mask_ref() -> np.ndarray:
    return np.triu(np.full((S, S), NEG, np.float32), k=1)


def _diag_mask_tiles() -> np.ndarray:
    p = np.arange(128, dtype=np.int64)[:, None]
    f = np.arange(SS, dtype=np.int64)[None, :]
    cols = [
        np.where(128 * j + p > f, np.float32(NEG), np.float32(0.0)) for j in range(4)
    ]
    return np.ascontiguousarray(np.concatenate(cols, axis=1).astype(np.float32))


def make_in_maps(hidden_states, attention_mask, cos, sin, wq, wk, wv, wo):
    """Host-side sharding/preprocessing. Returns (causal, in_maps)."""
    h = np.ascontiguousarray(np.asarray(hidden_states, dtype=np.float32)[0])
    m2 = np.ascontiguousarray(np.asarray(attention_mask, dtype=np.float32)[0, 0])
    wq = np.asarray(wq, dtype=np.float32)
    wk = np.asarray(wk, dtype=np.float32)
    wv = np.asarray(wv, dtype=np.float32)
    wo = np.asarray(wo, dtype=np.float32)

    causal = bool(np.array_equal(m2, _causal_mask_ref()))
    hT = np.ascontiguousarray(h.T)
    cosT = np.ascontiguousarray(np.asarray(cos, dtype=np.float32)[0].T)
    sinT = np.ascontiguousarray(np.asarray(sin, dtype=np.float32)[0].T)
    sc = np.float32(1.0 / math.sqrt(HD))
    if causal:
        md = _diag_mask_tiles()
    else:
        mT = np.ascontiguousarray(m2.T)

    in_maps = []
    for c in range(NCORES):
        im = {
            "hT": hT,
            "cosT": cosT,
            "sinT": sinT,
            "wqT": np.ascontiguousarray((wq[c * QH * HD : (c + 1) * QH * HD] * sc).T),
            "wkT": np.ascontiguousarray(wk[c * HD : (c + 1) * HD].T),
            "wvT": np.ascontiguousarray(wv[c * HD : (c + 1) * HD].T),
            "woT": np.ascontiguousarray(wo[:, c * QH * HD : (c + 1) * QH * HD].T),
        }
        if causal:
            im["maskd"] = md
        else:
            im["maskT"] = mT
        in_maps.append(im)
    return causal, in_maps


def kernel(hidden_states, attention_mask, cos, sin, wq, wk, wv, wo):
    causal, in_maps = make_in_maps(
        hidden_states, attention_mask, cos, sin, wq, wk, wv, wo
    )
    nc = _build(causal)
    res = run_bass_kernel_spmd(nc, in_maps, list(range(NCORES)))
    out = np.zeros((S, HID), np.float64)
    for c in range(NCORES):
        out += res.results[c]["y"].astype(np.float64)
    return out.reshape(B, S, HID).astype(np.float32)
